# revision 27
# baseline (speedup 1.0000x reference)
"""Multi-head attention block (nn_Attention) on 8 Trainium2 NeuronCores — v2.

Reference (fp32): qkv = x @ w_qkv; per-head softmax(q k^T / 8) v; out @ w_out + b.
Shapes: x [4,2048,1024], w_qkv [1024,3072], w_out [1024,1024], b_out [1024].

Sharding: DP over batch (4) x TP over head-groups (2) = 8 cores; core c does
batch c//2, heads [8*(c%2), 8*(c%2)+8). Host sums the two TP partials per batch
and adds b_out.

v2 design (vs v1 at ~520us):
- x is passed from the host pre-cast to fp16; x^T lands in SBUF via XBAR
  DMA-transposes straight from DRAM (no PE transposes, no PSUM copies).
- All 4 QK head-pair projections are fp16 and woven into the attention loop's
  spare PE cycles via a work queue; V likewise during the first i-block.
- exp split across two engines: ScalarE does 12/16 j-blocks per i-block
  (spline exp, exact); DVE does 4/16 with a two-point Schraudolph in fp16 bit
  space: u = int16(S*A + B); u2 = u + 502; P = f16bits(u) + f16bits(u2).
  Mean ratio ~1 (bias folded into B), variation < +-1.3% on ~25% of softmax
  mass -> ~3e-3 output error. One PSUM read per tile; int ops run at 2-4x.
- softmax denominators via the ones-column of V_aug (row 64 of the PV psum);
  normalization: reciprocal_approx_fast (DVE) + partition_broadcast (GpSimd)
  + one PSUM-direct multiply (DVE) writing oT in fp16.
- out-projection (fp16 oT x fp16 wo) interleaved into the last head-pair's
  attention via the same work queue; outputs DMA out per 128x512 block.

Engine budget/core: PE ~320us (S/PV 218 irreducible at d=64, QKV+V 82, tail 27),
Scalar ~250us, DVE ~230us, GpSimd ~100us.
"""
import sys

sys.path.insert(0, "/opt/trn_rl_repo")

import numpy as np

import concourse.bacc as bacc
import concourse.mybir as mybir
from concourse.tile import TileContext
from concourse.bass_utils import run_bass_kernel_spmd

F32 = mybir.dt.float32
F32R = mybir.dt.float32r
F16 = mybir.dt.float16
I16 = mybir.dt.int16
EXP = mybir.ActivationFunctionType.Exp
ADD = mybir.AluOpType.add
MULT = mybir.AluOpType.mult

T = 2048      # tokens per core (one batch element)
E = 1024      # model dim
D = 64        # head dim
SCALE = D ** -0.5
NEC = E // 128   # 8 E-chunks
NI = 4           # i-blocks of 512 queries
NJ = 16          # j-blocks of 128 keys

# fp16-bitspace Schraudolph constants (see module docstring); SCALE folded in.
A_EXP = float(1024 * np.log2(np.e) * SCALE)
B_EXP = 14011.875
D_EXP = 502.0
DVE_JBS = (1, 4, 7, 10, 13)   # j-block slots exp'd on DVE (5 of 16)

_CACHED_NC = None


def build_nc():
    nc = bacc.Bacc("TRN2", target_bir_lowering=False, debug=False, num_devices=8)
    x16t_d = nc.declare_dram_parameter("x16t", [E, T], F16, isOutput=False)
    wqk_d = nc.declare_dram_parameter("wqk", [E, 1024], F16, isOutput=False)
    wv_d = nc.declare_dram_parameter("wv", [E, 512], F16, isOutput=False)
    wo_d = nc.declare_dram_parameter("wo", [512, E], F16, isOutput=False)
    out_d = nc.declare_dram_parameter("out", [T, E], F32, isOutput=True)

    with TileContext(nc) as tc:
        with (
            tc.tile_pool(name="xt", bufs=1) as xt_pool,
            tc.tile_pool(name="qk", bufs=1) as qk_pool,
            tc.tile_pool(name="va", bufs=1) as va_pool,
            tc.tile_pool(name="w", bufs=1) as w_pool,
            tc.tile_pool(name="ot", bufs=1) as ot_pool,
            tc.tile_pool(name="pt", bufs=6) as p_pool,
            tc.tile_pool(name="ut", bufs=4) as u_pool,
            tc.tile_pool(name="rt", bufs=4) as r_pool,
            tc.tile_pool(name="ost", bufs=2) as o_pool,
            tc.tile_pool(name="s_ps", bufs=3, space="PSUM") as s_psum,
            tc.tile_pool(name="oa_ps", bufs=2, space="PSUM") as oa_psum,
        ):
            xT = xt_pool.tile([128, NEC * T], F16, tag="xT")
            xTv = xT[:].rearrange("p (ec t) -> p ec t", t=T)
            qp = [qk_pool.tile([128, T], F16, tag=f"qp{i}", name=f"qp{i}")
                  for i in range(4)]
            kp = [qk_pool.tile([128, T], F16, tag=f"kp{i}", name=f"kp{i}")
                  for i in range(4)]
            vaug = [va_pool.tile([128, 8 * 128], F16, tag=f"va{jb}", name=f"va{jb}")
                    for jb in range(NJ)]
            wq = [w_pool.tile([128, E], F16, tag=f"wq{i}", name=f"wq{i}")
                  for i in range(4)]
            wk = [w_pool.tile([128, E], F16, tag=f"wk{i}", name=f"wk{i}")
                  for i in range(4)]
            wv_sb = w_pool.tile([128, NEC * 512], F16, tag="wv")
            wo_sb = [w_pool.tile([128, E], F16, tag=f"wo{i}", name=f"wo{i}")
                     for i in range(4)]
            oT = [ot_pool.tile([128, T], F16, tag=f"oT{i}", name=f"oT{i}")
                  for i in range(4)]

            # ---------- DMAs (critical-path order) ---------------------------
            def load_xt(th):
                for sub in range(2):
                    tcols = slice(th * 512 + sub * 256, th * 512 + sub * 256 + 256)
                    nc.sync.dma_start(
                        out=xTv[:, :, tcols],
                        in_=x16t_d[:, tcols].rearrange("(ec p) t -> p ec t", p=128),
                    )

            def load_w_pair(pair):
                for w_sb, base in ((wq[pair], pair * 128),
                                   (wk[pair], 512 + pair * 128)):
                    nc.gpsimd.dma_start(
                        out=w_sb[:].rearrange("p (ec c) -> p ec c", c=128),
                        in_=wqk_d[:, base:base + 128].rearrange(
                            "(ec p) c -> p ec c", p=128),
                    )

            load_xt(0)
            for ec in range(NEC):   # wv via Scalar HWDGE (idle in prologue)
                nc.scalar.dma_start(
                    out=wv_sb[:, ec * 512:(ec + 1) * 512],
                    in_=wv_d[ec * 128:(ec + 1) * 128, :],
                )
            load_w_pair(0)
            for th in range(1, 4):
                load_xt(th)
            for pair in range(1, 4):
                load_w_pair(pair)
            for hc in range(4):
                nc.gpsimd.dma_start(
                    out=wo_sb[hc][:],
                    in_=wo_d[hc * 128:(hc + 1) * 128, :],
                )

            # ---------- deferred-work closures ([128,1024] s-pool tiles) ----
            def v_group2(tbp):
                tb0, tb1 = 2 * tbp, 2 * tbp + 1
                def fn():
                    ps = s_psum.tile([128, 1024], F32, tag="sAB", name=f"v{tbp}")
                    for ec in range(NEC):
                        nc.tensor.matmul(
                            ps[:, 0:512], xTv[:, ec, tb0 * 128:(tb0 + 1) * 128],
                            wv_sb[:, ec * 512:(ec + 1) * 512],
                            start=(ec == 0), stop=(ec == NEC - 1),
                        )
                        nc.tensor.matmul(
                            ps[:, 512:1024], xTv[:, ec, tb1 * 128:(tb1 + 1) * 128],
                            wv_sb[:, ec * 512:(ec + 1) * 512],
                            start=(ec == 0), stop=(ec == NEC - 1),
                        )
                    for half, tb in ((0, tb0), (1, tb1)):
                        vview = vaug[tb][:].rearrange("p (h c) -> p h c", c=128)
                        nc.vector.memset(vview[:, :, 0:64], 0.0)
                        nc.vector.memset(vview[:, :, 0:1], 1.0)
                        nc.scalar.activation(
                            vview[:, :, 64:128],
                            ps[:, half * 512:(half + 1) * 512].rearrange(
                                "p (h c) -> p h c", c=64),
                            mybir.ActivationFunctionType.Copy,
                        )
                return fn

            def qk_group1(w_sb, dst, ib):
                def fn():
                    ps = s_psum.tile([128, 1024], F32, tag="sAB",
                                     name=f"qk1_{ib}")
                    for ec in range(NEC):
                        wsl = w_sb[:, ec * 128:(ec + 1) * 128]
                        nc.tensor.matmul(
                            ps[:, 0:512], wsl,
                            xTv[:, ec, ib * 512:(ib + 1) * 512],
                            start=(ec == 0), stop=(ec == NEC - 1),
                        )
                    nc.scalar.copy(dst[:, ib * 512:(ib + 1) * 512],
                                   ps[:, 0:512])
                return fn

            def qk_group2(pair, w_sb, dst, ibp):
                ib0, ib1 = 2 * ibp, 2 * ibp + 1
                def fn():
                    ps = s_psum.tile([128, 1024], F32, tag="sAB",
                                     name=f"qk{pair}_{ibp}")
                    for ec in range(NEC):
                        wsl = w_sb[:, ec * 128:(ec + 1) * 128]
                        nc.tensor.matmul(
                            ps[:, 0:512], wsl,
                            xTv[:, ec, ib0 * 512:(ib0 + 1) * 512],
                            start=(ec == 0), stop=(ec == NEC - 1),
                        )
                        nc.tensor.matmul(
                            ps[:, 512:1024], wsl,
                            xTv[:, ec, ib1 * 512:(ib1 + 1) * 512],
                            start=(ec == 0), stop=(ec == NEC - 1),
                        )
                    nc.scalar.copy(dst[:, ib0 * 512:(ib0 + 2) * 512], ps[:])
                return fn

            def tail_block2(tb):
                def fn():
                    trows = slice(tb * 128, (tb + 1) * 128)
                    ps = s_psum.tile([128, 1024], F32, tag="sAB", name=f"t{tb}")
                    for hc in range(4):
                        osl = oT[hc][:, trows]
                        nc.tensor.matmul(
                            ps[:, 0:512], osl, wo_sb[hc][:, 0:512],
                            start=(hc == 0), stop=(hc == 3),
                        )
                        nc.tensor.matmul(
                            ps[:, 512:1024], osl, wo_sb[hc][:, 512:1024],
                            start=(hc == 0), stop=(hc == 3),
                        )
                    ot_sb = o_pool.tile([128, 1024], F32, tag="ost")
                    nc.vector.tensor_copy(ot_sb[:], ps[:])
                    deng = nc.sync if tb % 2 == 0 else nc.gpsimd
                    deng.dma_start(out=out_d[trows, :], in_=ot_sb[:])
                return fn

            work = []        # FIFO of deferred closures (PE-heavy)
            norm_work = []   # deferred normalization tails (DVE/GpSimd)

            # ---------- prologue: minimal gate for (hc0, ib0, jb0..1) -------
            v_group2(0)()
            qk_group1(wq[0], qp[0], 0)()
            qk_group1(wk[0], kp[0], 0)()

            # remaining pair0 groups woven into hc0-ib0 directly (see below);
            # pairs 1..3 go through the generic work queue.
            for pair in range(1, 4):
                work.append(qk_group2(pair, wq[pair], qp[pair], 0))
                work.append(qk_group2(pair, wq[pair], qp[pair], 1))
                work.append(qk_group2(pair, wk[pair], kp[pair], 0))
                work.append(qk_group2(pair, wk[pair], kp[pair], 1))

            # ---------- attention ------------------------------------------
            for hc in range(4):
                hA, hB = 2 * hc, 2 * hc + 1
                for ib in range(NI):
                    icols = slice(ib * 512, (ib + 1) * 512)
                    oaugA = oa_psum.tile([128, 512], F32, tag="oa", name="oaugA")
                    oaugB = oa_psum.tile([128, 512], F32, tag="oa", name="oaugB")
                    pv_emitted = [0]
                    pv_queue = []   # (run_fn, due_step)

                    def make_pv(jb, pAB, oaugA=oaugA, oaugB=oaugB, hA=hA, hB=hB):
                        def run(last):
                            first = pv_emitted[0] == 0
                            pv_emitted[0] += 1
                            nc.tensor.matmul(
                                oaugA[:], vaug[jb][:, hA * 128:hA * 128 + 128],
                                pAB[:, 0:512], start=first, stop=last,
                            )
                            nc.tensor.matmul(
                                oaugB[:], vaug[jb][:, hB * 128:hB * 128 + 128],
                                pAB[:, 512:1024], start=first, stop=last,
                            )
                        return run

                    for jb in range(NJ):
                        jcols = slice(jb * 128, (jb + 1) * 128)
                        sAB = s_psum.tile([128, 1024], F32, tag="sAB")
                        nc.tensor.matmul(
                            sAB[:, 0:512], kp[hc][0:64, jcols],
                            qp[hc][0:64, icols], start=True, stop=True,
                        )
                        nc.tensor.matmul(
                            sAB[:, 512:1024], kp[hc][64:128, jcols],
                            qp[hc][64:128, icols], start=True, stop=True,
                        )
                        pAB = p_pool.tile([128, 1024], F16, tag="pAB")
                        if jb in DVE_JBS:
                            u = u_pool.tile([128, 1024], I16, tag="u")
                            nc.vector.tensor_scalar(
                                u[:], sAB[:], A_EXP, B_EXP, MULT, ADD)
                            u2 = u_pool.tile([128, 1024], I16, tag="u2")
                            nc.vector.tensor_scalar(u2[:], u[:], D_EXP, None, ADD)
                            nc.vector.tensor_tensor(
                                pAB[:], u[:].bitcast(F16), u2[:].bitcast(F16), ADD)
                            delay = 4
                        else:
                            nc.scalar.activation(pAB[:], sAB[:], EXP, scale=SCALE)
                            delay = 1
                        pv_queue.append((make_pv(jb, pAB[:]), jb + delay))

                        # flush due PVs (keep at least the non-due ones queued)
                        while pv_queue and pv_queue[0][1] <= jb:
                            fn, _ = pv_queue.pop(0)
                            fn(False)

                        # weave in head-phase / tail work
                        step = jb
                        if hc == 0 and ib == 0:
                            if step == 0:
                                v_group2(1)()
                            if step == 2:
                                qk_group1(wk[0], kp[0], 1)()
                            if step in (3, 5, 7, 9, 11, 12):
                                v_group2({3: 2, 5: 3, 7: 4, 9: 5, 11: 6,
                                          12: 7}[step])()
                            if step == 6:
                                qk_group2(0, wk[0], kp[0], 1)()
                            if step == 13:
                                qk_group1(wq[0], qp[0], 1)()
                            if step == 14:
                                qk_group2(0, wq[0], qp[0], 1)()
                        else:
                            if hc == 3:
                                if work and step >= 8 and step % 2 == 0:
                                    work.pop(0)()
                            elif work and step % 5 == 2:
                                work.pop(0)()
                        if norm_work and step in (3, 7):
                            norm_work.pop(0)()

                    # flush remaining PVs; last one closes the group
                    while pv_queue:
                        fn, _ = pv_queue.pop(0)
                        fn(not pv_queue)



                    # ---------- normalization -> oT (fp16) -----------------
                    for oaug, rowoff in ((oaugA, 0), (oaugB, 64)):
                        oc = r_pool.tile([128, 512], F32, tag="oc")
                        if rowoff == 0:
                            nc.scalar.copy(oc[:], oaug[:])
                        else:
                            nc.vector.tensor_copy(oc[:], oaug[:])

                        def norm_tail(oc=oc, hc=hc, rowoff=rowoff, icols=icols):
                            rc = r_pool.tile([1, 512], F32, tag="rc")
                            nc.vector.reciprocal_approx_fast(rc[:], oc[0:1, :])
                            rbs = r_pool.tile([128, 512], F32, tag="rbs")
                            nc.gpsimd.partition_broadcast(rbs[:], rc[0:1, :])
                            nc.vector.tensor_tensor(
                                oT[hc][rowoff:rowoff + 64, icols],
                                oc[64:128, :], rbs[64:128, :], MULT)
                        norm_work.append(norm_tail)

                    if hc == 3:
                        for tb in range(ib * 4, ib * 4 + 4):
                            work.append(tail_block2(tb))

            # ---------- drain ----------------------------------------------
            for fn in norm_work:
                fn()
            norm_work = []
            for fn in work:
                fn()

    nc.compile()
    return nc


def get_nc():
    global _CACHED_NC
    if _CACHED_NC is None:
        _CACHED_NC = build_nc()
    return _CACHED_NC


def make_in_maps(x, w_qkv, w_out):
    in_maps = []
    for c in range(8):
        bi, hg = divmod(c, 2)
        wqk_c = np.concatenate(
            [
                w_qkv[:, hg * 512: hg * 512 + 512],
                w_qkv[:, 1024 + hg * 512: 1024 + hg * 512 + 512],
            ],
            axis=1,
        )
        in_maps.append(
            {
                "x16t": np.ascontiguousarray(x[bi].T.astype(np.float16)),
                "wqk": np.ascontiguousarray(wqk_c).astype(np.float16),
                "wv": np.ascontiguousarray(
                    w_qkv[:, 2048 + hg * 512: 2048 + hg * 512 + 512]
                ).astype(np.float16),
                "wo": np.ascontiguousarray(
                    w_out[hg * 512: hg * 512 + 512, :]
                ).astype(np.float16),
            }
        )
    return in_maps


def kernel(x, w_qkv, w_out, b_out):
    x = np.asarray(x, dtype=np.float32)
    w_qkv = np.asarray(w_qkv, dtype=np.float32)
    w_out = np.asarray(w_out, dtype=np.float32)
    b_out = np.asarray(b_out, dtype=np.float32)
    nc = get_nc()
    res = run_bass_kernel_spmd(nc, make_in_maps(x, w_qkv, w_out), list(range(8)))
    parts = [res.results[c]["out"] for c in range(8)]
    out = np.stack([parts[2 * bi] + parts[2 * bi + 1] for bi in range(4)])
    out += b_out[None, None, :]
    return out.astype(np.float32)
